# revision 58
# baseline (speedup 1.0000x reference)
"""Trainium2 Bass kernel for per-head Llama GQA attention.

Model: H=16 q heads, HKV=4 kv heads, head_dim=128, L=2048, D=2048, B=1.
Per-head hidden streams and per-head outputs (no cross-head reduction), so
tensor-parallel over heads is embarrassingly parallel: core c owns q heads
{2c, 2c+1} and their kv head c//2.  No collectives.

Per-core dataflow (v2 -- fully streamed/overlapped):
  - hidden streams arrive host-transposed as X^T in 512-column groups
    [NCH, 128, NDT, 512] fp16 so the 16 d-tile matmuls of one output chunk
    start as soon as that c-group lands; stream order (k || v), q0, q1 lets
    head-0 attention overlap head-1's DMA
  - all projections are weight-stationary (lhsT = W^T d-tile, rhs = x
    chunk, N=512) so LDWEIGHTS hides under the moving stream;  V is
    projected to (HD, L) and flipped to (L, HD) with four 128x128 PE
    transposes per chunk
  - RoPE in (HD, L): rotate-half via signed-permutation matmul; cos/sin
    are fp16 host tables shared by q and k (1/sqrt(HD) folded into wq)
  - causal masking at triangle granularity: diagonal 512-blocks run
    shrunk-width score/attn/ones matmuls (N = 512-128r) and only the
    128x128 diagonal triangles get an additive mask (one shared tile)
  - softmax denominator: ones-matmul column sums, basis-vector matmul to
    a per-partition column, reciprocal folded into the out-proj drain
  - out-projection of chunk c is interleaved piecewise into the attention
    stages of chunk c+1: PSUM slots recycle without stalling PE, and the
    PSUM->SBUF drains alternate DVE/ACT while stores issue on sync/gpsimd
"""

import os
import sys

sys.path.insert(0, "/opt/trn_rl_repo")

import numpy as np

import concourse.bass as bass
import concourse.tile as tile
from concourse import bacc, mybir
from concourse.bass_utils import run_bass_kernel_spmd

H, HKV, D, HD, L = 16, 4, 2048, 128, 2048
THETA = 10000.0
NC = 8
HPC = H // NC  # q heads per core (2)
NDT = D // 128  # d-tiles (16)
NLT = L // 128  # l/j tiles (16)
NCH = L // 512  # 512-wide chunks (4)
F16 = mybir.dt.float16
F32 = mybir.dt.float32
EXP = mybir.ActivationFunctionType.Exp
CPY = mybir.ActivationFunctionType.Copy

last_exec_time_ns = None
last_mean_exec_time_ns = None

_programs = {}


def _build_program(mode: str, debug: bool = False) -> bass.Bass:
    # Bacc (not plain Bass): its compile() runs the wait-splitting passes
    # (generate_event_semaphores) that walrus requires.
    nc = bacc.Bacc(None, target_bir_lowering=False)

    xq = nc.dram_tensor("xq", [HPC, NCH, 128, NDT, 512], F16, kind="ExternalInput")
    xk = nc.dram_tensor("xk", [NCH, 128, NDT, 512], F16, kind="ExternalInput")
    xv = nc.dram_tensor("xv", [NCH, 128, NDT, 512], F16, kind="ExternalInput")
    wq = nc.dram_tensor("wq", [128, HPC, NDT, 128], F16, kind="ExternalInput")
    wk = nc.dram_tensor("wk", [128, NDT, 128], F16, kind="ExternalInput")
    wv = nc.dram_tensor("wv", [128, NDT, 128], F16, kind="ExternalInput")
    wo = nc.dram_tensor("wo", [128, HPC, D], F16, kind="ExternalInput")
    cos = nc.dram_tensor("cos", [128, L], F16, kind="ExternalInput")
    sin = nc.dram_tensor("sin", [128, L], F16, kind="ExternalInput")
    # misc fp16: [:, 0:128] rotate-half perm (lhsT), [:, 128:256] ones,
    # [:, 256:384] identity (for PE transposes)
    misc = nc.dram_tensor("misc", [128, 384], F16, kind="ExternalInput")
    e1 = nc.dram_tensor("e1", [128, 1], F16, kind="ExternalInput")
    if mode == "causal":
        # one shared 128x128 additive triangle (0 where j<=l else -1e9)
        mtri = nc.dram_tensor("mtri", [128, 128], F32, kind="ExternalInput")
    elif mode == "general":
        # j-tile-pair packed additive mask, fp16
        maskg = nc.dram_tensor("maskg", [128, NLT // 2, NCH, 1024], F16,
                               kind="ExternalInput")
    out = nc.dram_tensor("out", [HPC, NLT, 128, D], F16, kind="ExternalOutput")
    if debug:
        dbg_qrot = nc.dram_tensor("dbg_qrot", [HPC, 128, L], F16, kind="ExternalOutput")
        dbg_krot = nc.dram_tensor("dbg_krot", [128, L], F16, kind="ExternalOutput")
        dbg_v16 = nc.dram_tensor("dbg_v16", [128, L], F16, kind="ExternalOutput")
        dbg_attn = nc.dram_tensor("dbg_attn", [HPC, NCH, 128, 512], F16, kind="ExternalOutput")
        dbg_sums = nc.dram_tensor("dbg_sums", [HPC, NCH, 128, 512], F32, kind="ExternalOutput")

    with tile.TileContext(nc) as tc:
        with (
            tc.tile_pool(name="const", bufs=1) as constp,
            tc.tile_pool(name="xs", bufs=6) as xpool,
            tc.tile_pool(name="persist", bufs=1) as persist,
            tc.tile_pool(name="probs", bufs=10 if mode != "general" else 7) as probsp,
            tc.tile_pool(name="ropetmp", bufs=4) as ropep,
            tc.tile_pool(name="vt", bufs=2) as vtp,
            tc.tile_pool(name="sums", bufs=4) as sumsp,
            tc.tile_pool(name="att16", bufs=4) as att16p,
            tc.tile_pool(name="recs", bufs=6) as recs,
            tc.tile_pool(name="outs", bufs=5) as outsp,
            # PSUM: 8 banks total.
            #   pA: 2 x (128,1024) f32 (2 banks each) - score pairs, out-proj
            #       pairs, rotate-half, pdiag, phase-A spill
            #   pB: 2 x (128,512) f32 (1 bank each) - pattn + ones accumulators
            #       (phase A: packed V-transpose tiles)
            #   pP: 2 x (128,512) f32 - projection accumulators (k+v / q)
            tc.tile_pool(name="pA", bufs=2, space="PSUM") as pA,
            tc.tile_pool(name="pB", bufs=2, space="PSUM") as pB,
            tc.tile_pool(name="pP", bufs=2, space="PSUM") as pP,
        ):
            # ---- constants (projection weights first so PE starts early;
            # wo is issued later from the scalar queue -- it is not needed
            # until the first out-projection and its 1MB would otherwise
            # steal HBM bandwidth from the critical first x chunks) ----
            wk_sb = constp.tile([128, NDT, 128], F16, tag="wk")
            nc.sync.dma_start(out=wk_sb[:], in_=wk[:])
            wv_sb = constp.tile([128, NDT, 128], F16, tag="wv")
            nc.sync.dma_start(out=wv_sb[:], in_=wv[:])
            misc_sb = constp.tile([128, 384], F16, tag="misc")
            nc.sync.dma_start(out=misc_sb[:], in_=misc[:])
            w_all = constp.tile([128, HPC, NDT, 128], F16, tag="wq")
            nc.sync.dma_start(out=w_all[:], in_=wq[:])
            cos_sb = constp.tile([128, L], F16, tag="cos")
            nc.sync.dma_start(out=cos_sb[:], in_=cos[:])
            sin_sb = constp.tile([128, L], F16, tag="sin")
            nc.sync.dma_start(out=sin_sb[:], in_=sin[:])
            e1_sb = constp.tile([128, 1], F16, tag="e1")
            nc.sync.dma_start(out=e1_sb[:], in_=e1[:])
            wo_sb = constp.tile([128, HPC, D], F16, tag="wo")
            nc.sync.dma_start(out=wo_sb[:], in_=wo[:])
            perm = misc_sb[:, 0:128]
            ones = misc_sb[:, 128:256]
            ident = misc_sb[:, 256:384]
            if mode == "causal":
                mtri_sb = constp.tile([128, 128], F32, tag="mtri")
                nc.sync.dma_start(out=mtri_sb[:], in_=mtri[:])

            # persistent per-core activations
            krot = persist.tile([128, L], F16, tag="krot")
            v16 = persist.tile([128, L], F16, tag="v16")
            qrot = [
                persist.tile([128, L], F16, tag=f"qrot{i}", name=f"qrot{i}")
                for i in range(HPC)
            ]

            def x_group(x_dram_c, eng, pieces=4):
                """DMA one 512-column group [128, NDT, 512] in pieces so
                projection matmuls start early and PE gaps stay well under
                the ~3.4us HAM re-throttle window."""
                xt = xpool.tile([128, NDT, 512], F16, tag="xtile")
                w = NDT // pieces
                for p in range(pieces):
                    eng.dma_start(
                        out=xt[:, w * p : w * p + w, :],
                        in_=x_dram_c[:, w * p : w * p + w, :],
                    )
                return xt

            def proj_chunk(xt, w_sb):
                """One 512-wide projection chunk: accumulate 16 d-tiles."""
                pp = pP.tile([128, 512], F32, tag="pP")
                for dt in range(NDT):
                    nc.tensor.matmul(
                        pp[:],
                        w_sb[:, dt, :],
                        xt[:, dt, :],
                        start=(dt == 0),
                        stop=(dt == NDT - 1),
                    )
                return pp

            def proj_chunk2(xt_a, w_a, xt_b, w_b, warm=False):
                """Two interleaved projection chunks (k and v arrive on
                parallel queues; alternate 4-MM bursts between them).  With
                warm=True a couple of throwaway matmuls ride along per piece
                to hold PE activity above the HAM re-throttle threshold
                while the stream is DMA-bound."""
                pa = pP.tile([128, 512], F32, tag="pP", name="pka")
                pb = pP.tile([128, 512], F32, tag="pP", name="pkb")
                for p in range(4):
                    for pp, xt, w_sb in ((pa, xt_a, w_a), (pb, xt_b, w_b)):
                        for dt in range(4 * p, 4 * p + 4):
                            nc.tensor.matmul(
                                pp[:],
                                w_sb[:, dt, :],
                                xt[:, dt, :],
                                start=(dt == 0),
                                stop=(dt == NDT - 1),
                            )
                    if warm:
                        pw = pA.tile([128, 512], F32, tag="pA", name=f"wp{p}")
                        for rr in range(2):
                            nc.tensor.matmul(
                                pw[:], w_a[:, 4 * p, :], xt_a[:, 4 * p, :]
                            )
                return pa, pb

            def rope_chunk(pp, c, dst):
                """RoPE on a (128,512) projected chunk -> dst fp16 slice."""
                sl = slice(c * 512, (c + 1) * 512)
                u16 = ropep.tile([128, 512], F16, tag="u16")
                nc.scalar.copy(out=u16[:], in_=pp[:])
                rh = pA.tile([128, 1024], F32, tag="pA")
                nc.tensor.matmul(rh[:, 0:512], perm, u16[:])
                t0 = ropep.tile([128, 512], F16, tag="t0")
                nc.vector.tensor_mul(out=t0[:], in0=u16[:], in1=cos_sb[:, sl])
                nc.vector.tensor_mul(out=dst[:, sl], in0=rh[:, 0:512], in1=sin_sb[:, sl])
                nc.vector.tensor_add(out=dst[:, sl], in0=dst[:, sl], in1=t0[:])

            def v_chunk(pp, c):
                """Drain V chunk (HD,512) and transpose to v16 (L,HD) blocks."""
                vt16 = vtp.tile([128, 512], F16, tag="vt16")
                nc.scalar.copy(out=vt16[:], in_=pp[:])
                ptr = pB.tile([128, 512], F16, tag="pB")
                for r in range(4):
                    nc.tensor.transpose(
                        ptr[:, r * 128 : (r + 1) * 128],
                        vt16[:, r * 128 : (r + 1) * 128],
                        ident,
                    )
                nc.vector.tensor_copy(
                    out=v16[:, c * 512 : (c + 1) * 512], in_=ptr[:]
                )

            drain_ctr = [0]

            # ---------------- attention ----------------
            # Each (head, chunk) builds a list of stages; each stage emits its
            # score matmuls + exp, then the PREVIOUS stage's attn/ones matmuls,
            # and one pending out-projection piece of the previous chunk --
            # keeping PE dense while PSUM slots recycle behind exp/drains.
            pending = []  # out-proj closures from the previous (head, chunk)

            def attn_chunk(i, c):
                # the final super-group has no later projections, so head 0's
                # accumulators can live in the (now idle) projection banks --
                # both last chunks then start without waiting on the previous
                # chunk's PSUM drain copies
                last0 = mode == "causal" and c == NCH - 1 and i == 0
                acc_pool = pP if last0 else pB
                acc_tag = "pP" if last0 else "pB"
                pattn = acc_pool.tile(
                    [128, 512], F32, tag=acc_tag, name=f"pattn{i}{c}"
                )
                pones = acc_pool.tile(
                    [128, 512], F32, tag=acc_tag, name=f"pones{i}{c}"
                )
                qsl = qrot[i][:, c * 512 : (c + 1) * 512]
                first = [True]
                stages = []

                njt_full = 4 * c if mode == "causal" else NLT

                def full_pair(jp):
                    jt0 = 2 * jp
                    st = {}

                    def scores():
                        sp = pA.tile([128, 1024], F32, tag="pA")
                        nc.tensor.matmul(
                            sp[:, 0:512], krot[:, jt0 * 128 : (jt0 + 1) * 128], qsl
                        )
                        nc.tensor.matmul(
                            sp[:, 512:1024],
                            krot[:, (jt0 + 1) * 128 : (jt0 + 2) * 128],
                            qsl,
                        )
                        if mode == "general":
                            mg = ropep.tile([128, 1024], F16, tag="maskg")
                            nc.gpsimd.dma_start(out=mg[:], in_=maskg[:, jp, c, :])
                            nc.vector.tensor_add(out=sp[:], in0=sp[:], in1=mg[:])
                        pe = probsp.tile([128, 1024], F16, tag="probs")
                        nc.scalar.activation(out=pe[:], in_=sp[:], func=EXP)
                        st["pe"] = pe

                    def consume(last):
                        pe = st["pe"]
                        nc.tensor.matmul(
                            pattn[:],
                            v16[:, jt0 * 128 : (jt0 + 1) * 128],
                            pe[:, 0:512],
                            start=first[0], stop=False,
                        )
                        nc.tensor.matmul(
                            pattn[:],
                            v16[:, (jt0 + 1) * 128 : (jt0 + 2) * 128],
                            pe[:, 512:1024],
                            start=False, stop=last,
                        )
                        nc.tensor.matmul(
                            pones[:], ones, pe[:, 0:512],
                            start=first[0], stop=False,
                        )
                        nc.tensor.matmul(
                            pones[:], ones, pe[:, 512:1024],
                            start=False, stop=last,
                        )
                        first[0] = False

                    return scores, consume

                def diag_pair(dp):
                    # diagonal 512-block, j-tiles r=2dp, 2dp+1 (within block).
                    # tile r covers l-columns [128r, 512) of the chunk; only
                    # its first 128 columns need the triangle mask.
                    rs = [2 * dp, 2 * dp + 1]
                    ns = [512 - 128 * r for r in rs]
                    offs = [0, ns[0]]
                    st = {}

                    def scores():
                        sp = pA.tile([128, 1024], F32, tag="pA")
                        for r, n, off in zip(rs, ns, offs):
                            jt = 4 * c + r
                            nc.tensor.matmul(
                                sp[:, off : off + n],
                                krot[:, jt * 128 : (jt + 1) * 128],
                                qrot[i][:, c * 512 + 128 * r : (c + 1) * 512],
                            )
                        for _, n, off in zip(rs, ns, offs):
                            nc.vector.tensor_add(
                                out=sp[:, off : off + 128],
                                in0=sp[:, off : off + 128],
                                in1=mtri_sb[:],
                            )
                        pe = probsp.tile([128, 1024], F16, tag="probs")
                        w = ns[0] + ns[1]
                        nc.scalar.activation(
                            out=pe[:, 0:w], in_=sp[:, 0:w], func=EXP
                        )
                        st["pe"] = pe

                    def consume(last):
                        pe = st["pe"]
                        for idx, (r, n, off) in enumerate(zip(rs, ns, offs)):
                            jt = 4 * c + r
                            is_last = last and idx == 1
                            nc.tensor.matmul(
                                pattn[:, 128 * r : 512],
                                v16[:, jt * 128 : (jt + 1) * 128],
                                pe[:, off : off + n],
                                start=first[0], stop=is_last,
                                skip_group_check=True,
                            )
                            nc.tensor.matmul(
                                pones[:, 128 * r : 512],
                                ones,
                                pe[:, off : off + n],
                                start=first[0], stop=is_last,
                                skip_group_check=True,
                            )
                            first[0] = False

                    return scores, consume

                for jp in range(njt_full // 2):
                    stages.append(full_pair(jp))
                if mode == "causal":
                    stages.append(diag_pair(0))
                    stages.append(diag_pair(1))

                # The first stage's scores are fully independent (qrot/krot
                # resident), so they go out BEFORE any pending out-proj
                # pieces -- the first pieces of a fresh chunk wait on that
                # chunk's attn16 copy and would stall PE at the boundary.
                # Remaining pieces are paced across the stages.
                nst = len(stages)
                prev = stages[0]
                prev[0]()
                for idx, stg in enumerate(stages[1:], start=1):
                    stg[0]()  # scores + exp
                    # out-proj filler lands between the scores and the
                    # consume so PE has work while ACT finishes the exp
                    quota = -(-len(pending) // (nst - idx)) if pending else 0
                    for _ in range(min(quota, 3)):
                        if pending:
                            pending.pop(0)()
                    prev[1](False)
                    prev = stg
                if nst == 1 and pending:
                    pending.pop(0)()
                prev[1](True)

                # free the accumulators quickly: row sums to SBUF (DVE),
                # unnormalized attnT to fp16 (ACT).  NOTE: a full-width DVE
                # reciprocal here costs 3.4us (iterative op) and stalls PE
                # into a HAM re-throttle -- keep the reciprocal tiny (128,4)
                # in pd_piece instead and scale during the drains.
                sums32 = sumsp.tile([128, 512], F16, tag="sums32")
                nc.vector.tensor_copy(out=sums32[:], in_=pones[:])
                attn16 = att16p.tile([128, 512], F16, tag="attn16")
                nc.scalar.copy(out=attn16[:], in_=pattn[:])
                if debug:
                    nc.sync.dma_start(out=dbg_sums[i, c], in_=sums32[:])
                    nc.sync.dma_start(out=dbg_attn[i, c], in_=attn16[:])
                return sums32, attn16

            def outproj_pieces(i, c, sums32, attn16, split_drain=False):
                """Closures: pdiag+tiny recip, then 8 out-proj pair pieces
                with the softmax reciprocal folded into the drains.
                split_drain=True (final chunk) drains each pair on BOTH
                engines in parallel and stores per pair, shortening the
                post-matmul tail."""
                cell = {}

                def pd_piece():
                    # basis-vector matmuls turn the broadcast row sums into a
                    # per-partition column; reciprocal on (128,4) is ~175ns
                    pd = pA.tile([128, 1024], F32, tag="pA")
                    for ls in range(4):
                        nc.tensor.matmul(
                            pd[:, ls : ls + 1],
                            sums32[:, ls * 128 : (ls + 1) * 128],
                            e1_sb[:],
                        )
                    recip = recs.tile([128, 4], F32, tag="recip")
                    nc.vector.reciprocal(out=recip[:], in_=pd[:, 0:4])
                    cell["recip"] = recip

                pieces = [pd_piece]
                for ls in range(4):
                    lt = 4 * c + ls
                    for dp in range(2):
                        def po_piece(ls=ls, lt=lt, dp=dp):
                            a_sl = attn16[:, ls * 128 : (ls + 1) * 128]
                            r_sl = cell["recip"][:, ls : ls + 1]
                            po = pA.tile([128, 1024], F32, tag="pA")
                            nc.tensor.matmul(
                                po[:, 0:512],
                                a_sl,
                                wo_sb[:, i, dp * 1024 : dp * 1024 + 512],
                            )
                            nc.tensor.matmul(
                                po[:, 512:1024],
                                a_sl,
                                wo_sb[:, i, dp * 1024 + 512 : dp * 1024 + 1024],
                            )
                            # single full-width drain per pair (one op
                            # amortizes the per-instruction bubble), ~3:2
                            # DVE:ACT so ACT keeps headroom for the exps;
                            # both pairs of an l-tile share one staging tile
                            # and one store DMA.
                            if split_drain:
                                ost = outsp.tile(
                                    [128, 2048], F16, tag="ost", name=f"oss{lt}"
                                )
                                nc.vector.tensor_scalar_mul(
                                    out=ost[:, 0:512], in0=po[:, 0:512],
                                    scalar1=r_sl,
                                )
                                nc.scalar.activation(
                                    out=ost[:, 512:1024], in_=po[:, 512:1024],
                                    func=CPY, scale=r_sl,
                                )
                                # alternate store queues: gpsimd is idle at
                                # the end of the kernel, halving the final
                                # store tail
                                seng = nc.sync if dp == 0 else nc.gpsimd
                                seng.dma_start(
                                    out=out[i, lt, :,
                                            dp * 1024 : (dp + 1) * 1024],
                                    in_=ost[:, 0:1024],
                                )
                                return
                            if dp == 0:
                                cell[ls] = outsp.tile(
                                    [128, 2048], F16, tag="ost", name=f"ost{lt}"
                                )
                            ost = cell[ls]
                            osl = ost[:, dp * 1024 : (dp + 1) * 1024]
                            drain_ctr[0] += 1
                            if drain_ctr[0] % 5 < 3:
                                nc.vector.tensor_scalar_mul(
                                    out=osl, in0=po[:], scalar1=r_sl
                                )
                            else:
                                nc.scalar.activation(
                                    out=osl, in_=po[:], func=CPY, scale=r_sl
                                )
                            if dp == 1:
                                nc.sync.dma_start(out=out[i, lt], in_=ost[:])
                        pieces.append(po_piece)
                return pieces

            # ---------------- emission ----------------
            # Causal: fully merged stream -- each c-group carries k, v, q0
            # AND q1 columns, and BOTH heads' attention chunks follow (chunk
            # c only attends to k/v columns <= c).  Each head's
            # out-projection fills the other head's attention stages, so PE
            # stays dense end to end.
            # Non-causal: attention chunk c reads ALL of krot/v16, so k and
            # v must be fully projected before any attention runs.
            def kv_cgroup(c):
                kt = xpool.tile([128, NDT, 512], F16, tag="xtile", name="kt")
                vt = xpool.tile([128, NDT, 512], F16, tag="xtile", name="vt")
                np_ = 8 if c == 0 else 4
                w = NDT // np_
                for p in range(np_):
                    sl = slice(w * p, w * p + w)
                    nc.gpsimd.dma_start(out=kt[:, sl, :], in_=xk[c][:, sl, :])
                    nc.gpsimd.dma_start(out=vt[:, sl, :], in_=xv[c][:, sl, :])
                if c == 0:
                    # HAM warm-up: the first c-group is DMA-bound, and the
                    # sparse matmul stream would leave the PE clock gated at
                    # 1.2GHz well into the first attention chunks.  A burst
                    # of throwaway matmuls on the first k piece raises the
                    # activity ratio past the un-throttle threshold; their
                    # results land in rotating pA tiles nothing reads.
                    for r in range(6):
                        pw = pA.tile([128, 512], F32, tag="pA", name=f"warm{r}")
                        for rr in range(4):
                            nc.tensor.matmul(
                                pw[:], wk_sb[:, rr % 2, :], kt[:, rr % 2, :]
                            )
                pk, pv = proj_chunk2(kt, wk_sb, vt, wv_sb, warm=(c < 2))
                rope_chunk(pk, c, krot)
                return pv

            def head_chunk(i, c, last=False):
                sums32, attn16 = attn_chunk(i, c)
                # keep at most one chunk's worth of leftovers queued so
                # tile pools stay shallow; the rest carries over as
                # filler for the next chunk's stages
                while len(pending) > 9:
                    pending.pop(0)()
                pending.extend(outproj_pieces(
                    i, c, sums32, attn16, split_drain=last,
                ))

            if mode == "causal":
                for c in range(NCH):
                    pv = kv_cgroup(c)
                    qt0 = x_group(xq[0, c], nc.gpsimd)
                    qt1 = x_group(xq[1, c], nc.gpsimd)
                    pq0 = proj_chunk(qt0, w_all[:, 0])
                    rope_chunk(pq0, c, qrot[0])
                    v_chunk(pv, c)
                    pq1 = proj_chunk(qt1, w_all[:, 1])
                    rope_chunk(pq1, c, qrot[1])
                    for i in range(HPC):
                        head_chunk(i, c, last=(i == HPC - 1 and c == NCH - 1))
            else:
                for c in range(NCH):
                    pv = kv_cgroup(c)
                    v_chunk(pv, c)
                for i in range(HPC):
                    for c in range(NCH):
                        qt = x_group(xq[i, c], nc.gpsimd)
                        pq = proj_chunk(qt, w_all[:, i])
                        rope_chunk(pq, c, qrot[i])
                        head_chunk(i, c,
                                   last=(i == HPC - 1 and c == NCH - 1))
            if debug:
                nc.sync.dma_start(out=dbg_krot[:], in_=krot[:])
                nc.sync.dma_start(out=dbg_v16[:], in_=v16[:])
                nc.sync.dma_start(out=dbg_qrot[0], in_=qrot[0][:])
                nc.sync.dma_start(out=dbg_qrot[1], in_=qrot[1][:])
            while pending:
                pending.pop(0)()
    nc.compile()
    return nc


def _get_program(mode: str) -> bass.Bass:
    debug = os.environ.get("KERNEL_DEBUG", "0") != "0"
    key = (mode, debug)
    if key not in _programs:
        _programs[key] = _build_program(mode, debug)
    return _programs[key]


def _rope_tables(position_ids: np.ndarray):
    pos = position_ids.reshape(-1).astype(np.float32)  # (L,)
    inv_freq = (
        1.0 / (THETA ** (np.arange(0, HD, 2, dtype=np.float32) / HD))
    ).astype(np.float32)
    freqs = pos[:, None] * inv_freq[None, :]  # (L, HD/2)
    emb = np.concatenate([freqs, freqs], axis=1)  # (L, HD)
    cos = np.ascontiguousarray(np.cos(emb).T).astype(np.float16)  # (HD, L)
    sin = np.ascontiguousarray(np.sin(emb).T).astype(np.float16)
    return cos, sin


def _x_groups(x):  # (L, D) fp32 -> (NCH, 128, NDT, 512) fp16 c-group tiles
    xt = x.T.astype(np.float16).reshape(NDT, 128, NCH, 512)
    return np.ascontiguousarray(xt.transpose(2, 1, 0, 3))


def kernel(
    q_hidden, k_hidden, v_hidden, wq, wk, wv, wo, attention_mask, position_ids
):
    global last_exec_time_ns, last_mean_exec_time_ns
    q_hidden = np.asarray(q_hidden)
    k_hidden = np.asarray(k_hidden)
    v_hidden = np.asarray(v_hidden)
    wq = np.asarray(wq, dtype=np.float32)
    wk = np.asarray(wk, dtype=np.float32)
    wv = np.asarray(wv, dtype=np.float32)
    wo = np.asarray(wo, dtype=np.float32)
    attention_mask = np.asarray(attention_mask, dtype=np.float32)
    position_ids = np.asarray(position_ids)

    mask2d = attention_mask.reshape(L, L)
    causal_ref = np.where(
        np.tril(np.ones((L, L), dtype=bool)), np.float32(0.0), np.float32(-1e9)
    )
    if np.array_equal(mask2d, causal_ref):
        mode = "causal"
    elif not mask2d.any():
        mode = "zero"
    else:
        mode = "general"

    cos, sin = _rope_tables(position_ids)
    scale = np.float32(1.0 / np.sqrt(HD))

    misc_h = np.zeros((128, 384), dtype=np.float16)
    # rotate-half: rh = P @ q with P[i, i+64] = -1 (i<64), P[i, i-64] = +1;
    # stored as lhsT = P^T
    for a in range(64):
        misc_h[a, a + 64] = np.float16(1.0)  # P^T[a, a+64] = P[a+64, a] = +1
        misc_h[a + 64, a] = np.float16(-1.0)  # P^T[a+64, a] = P[a, a+64] = -1
    misc_h[:, 128:256] = np.float16(1.0)  # ones block
    misc_h[np.arange(128), 256 + np.arange(128)] = np.float16(1.0)  # identity
    e1_h = np.zeros((128, 1), dtype=np.float16)
    e1_h[0, 0] = 1.0

    if mode == "causal":
        jj = np.arange(128, dtype=np.int32)[:, None]
        ll = np.arange(128, dtype=np.int32)[None, :]
        mtri_h = np.where(jj <= ll, np.float32(0.0), np.float32(-1e9)).astype(
            np.float32
        )
    elif mode == "general":
        # (128, NLT//2, NCH, 1024) fp16: pair jp holds j-tiles 2jp | 2jp+1
        mclip = np.clip(mask2d, -30000.0, 30000.0)
        mt = mclip.T.reshape(NLT, 128, NCH, 512)
        maskg_h = np.ascontiguousarray(
            np.concatenate([mt[0::2], mt[1::2]], axis=3).transpose(1, 0, 2, 3)
        ).astype(np.float16)

    wq_r = wq.reshape(H, HD, D) * scale  # fold 1/sqrt(HD) into wq
    wk_r = wk.reshape(HKV, HD, D)
    wv_r = wv.reshape(HKV, HD, D)
    wo_r = wo.reshape(D, H, HD)

    in_maps = []
    for core in range(NC):
        heads = [HPC * core + i for i in range(HPC)]
        g = heads[0] // (H // HKV)
        # weights: lhsT layout W^T tiles, partition-major
        wq_t = np.stack(
            [
                wq_r[n].T.astype(np.float16).reshape(NDT, 128, HD)
                for n in heads
            ],
            axis=0,
        )  # (HPC, NDT, 128p, 128m)
        wq_t = np.ascontiguousarray(wq_t.transpose(2, 0, 1, 3))  # (128, HPC, NDT, 128)
        wk_t = wk_r[g].T.astype(np.float16).reshape(NDT, 128, HD)
        wk_t = np.ascontiguousarray(wk_t.transpose(1, 0, 2))  # (128, NDT, 128)
        wv_t = wv_r[g].T.astype(np.float16).reshape(NDT, 128, HD)
        wv_t = np.ascontiguousarray(wv_t.transpose(1, 0, 2))
        wo_t = np.stack(
            [wo_r[:, n, :].T.astype(np.float16) for n in heads], axis=0
        )  # (HPC, 128, D)
        wo_t = np.ascontiguousarray(wo_t.transpose(1, 0, 2))  # (128, HPC, D)

        m = {
            "xq": np.stack([_x_groups(q_hidden[n, 0]) for n in heads], axis=0),
            "xk": _x_groups(k_hidden[g, 0]),
            "xv": _x_groups(v_hidden[g, 0]),
            "wq": wq_t,
            "wk": wk_t,
            "wv": wv_t,
            "wo": wo_t,
            "cos": cos,
            "sin": sin,
            "misc": misc_h,
            "e1": e1_h,
        }
        if mode == "causal":
            m["mtri"] = mtri_h
        elif mode == "general":
            m["maskg"] = maskg_h
        in_maps.append(m)

    nc = _get_program(mode)
    trace_env = os.environ.get("KERNEL_TRACE", "0")
    kwargs = {}
    if trace_env != "0":
        kwargs["trace"] = True
        if trace_env == "8":
            kwargs["trace_cores"] = list(range(NC))
    res = run_bass_kernel_spmd(nc, in_maps, core_ids=list(range(NC)), **kwargs)
    last_exec_time_ns = res.exec_time_ns
    last_mean_exec_time_ns = res.mean_exec_time_ns
    globals()["last_results"] = res.results
    globals()["last_in_maps"] = in_maps
    globals()["last_res"] = res

    out = np.empty((H, 1, L, D), dtype=np.float32)
    for core in range(NC):
        o = res.results[core]["out"]  # (HPC, NLT, 128, D) fp16
        for i in range(HPC):
            out[HPC * core + i, 0] = o[i].reshape(L, D).astype(np.float32)
    return out
